# revision 1
# baseline (speedup 1.0000x reference)
"""Self-contained Trainium2 Bass kernel for nn_Attention_62560493633940.

Sharding: 16 heads split across 8 cores (2 q-heads + their shared kv-head
per core, tensor parallel); x / pos replicated; per-core partial output
projections (over that core's 128 o-columns) summed on host.

Math note: pos_logits[h,q,k] = a[q,h] - a[k,h] + bh[h] with a = p @ Wh.T,
so softmax_k(pos_logits) is independent of q (shift invariance) ->
pos_attn is a rank-1 per-head key distribution; no [t,t,PF] diff tensor.
Both softmax row-sums are exactly 1, so the re-normalization in the
reference is an identity and the gate mix is (1-g)*attn + g*pos_attn.
"""
import sys

if '/opt/trn_rl_repo' not in sys.path:
    sys.path.insert(0, '/opt/trn_rl_repo')

import numpy as np

import concourse.bass as bass
import concourse.bacc as bacc
import concourse.tile as tile
import concourse.mybir as mybir
from concourse import bass_utils
from concourse.masks import make_identity

F32 = mybir.dt.float32
F16 = mybir.dt.float16

T = 1024      # sequence length
DIM = 1024    # model dim
H = 16        # heads
KVH = 4       # kv heads
HD = 64       # head dim
PD = 64       # pos dim
PF = 128      # pos feature dim
BASE = 10000.0
NC = 8        # cores

_CACHE = {}


def _build_program(reps=1):
    nc = bacc.Bacc("TRN2")

    # ---- DRAM parameters (per-core data arrives via in_maps) ----
    xT_d = nc.declare_dram_parameter("xT", [DIM, T], F16, isOutput=False)
    wq_d = nc.declare_dram_parameter("wq", [DIM, 128], F16, isOutput=False)
    wkv_d = nc.declare_dram_parameter("wkv", [DIM, 128], F16, isOutput=False)
    wo_d = nc.declare_dram_parameter("wo", [128, DIM], F16, isOutput=False)
    posT_d = nc.declare_dram_parameter("posT", [PD, T], F16, isOutput=False)
    wp1T_d = nc.declare_dram_parameter("wp1T", [PD, PD], F16, isOutput=False)
    wp2T_d = nc.declare_dram_parameter("wp2T", [PD, PF], F16, isOutput=False)
    whT2_d = nc.declare_dram_parameter("whT2", [PF, 32], F16, isOutput=False)
    bp1_d = nc.declare_dram_parameter("bp1c", [PD, 1], F32, isOutput=False)
    bp2_d = nc.declare_dram_parameter("bp2c", [PF, 1], F32, isOutput=False)
    g2a_d = nc.declare_dram_parameter("gate2a", [2, 1], F32, isOutput=False)
    g2b_d = nc.declare_dram_parameter("gate2b", [33, 1], F32, isOutput=False)
    tabc_d = nc.declare_dram_parameter("tabc", [32, T], F16, isOutput=False)
    tabs_d = nc.declare_dram_parameter("tabs", [64, T], F16, isOutput=False)
    outp_d = nc.declare_dram_parameter("outp", [T, DIM], F16, isOutput=True)

    ExpF = mybir.ActivationFunctionType.Exp
    ReluF = mybir.ActivationFunctionType.Relu
    IdentF = mybir.ActivationFunctionType.Identity
    SigF = mybir.ActivationFunctionType.Sigmoid
    CopyF = mybir.ActivationFunctionType.Copy

    with tile.TileContext(nc) as tc:
        with tc.tile_pool(name="cst", bufs=1) as cst, \
             tc.tile_pool(name="wk", bufs=1) as wk, \
             tc.tile_pool(name="eP", bufs=6) as eP, \
             tc.tile_pool(name="vP", bufs=8) as vP, \
             tc.tile_pool(name="psW", bufs=2, space="PSUM") as psW, \
             tc.tile_pool(name="psA", bufs=3, space="PSUM") as psA, \
             tc.tile_pool(name="psS", bufs=1, space="PSUM") as psS:

            for _rep in range(reps):
                # ---- constants into SBUF ----
                xT_sb = cst.tile([128, 8, T], F16)
                xT_ap = xT_d.ap().rearrange("(k p) t -> p k t", p=128)
                nc.gpsimd.dma_start(out=xT_sb[:, 0:1, 0:512],
                                    in_=xT_ap[:, 0:1, 0:512])
                nc.gpsimd.dma_start(out=xT_sb[:, 0:1, 512:1024],
                                    in_=xT_ap[:, 0:1, 512:1024])
                for kk in range(1, 8):
                    nc.gpsimd.dma_start(out=xT_sb[:, kk:kk + 1, :],
                                        in_=xT_ap[:, kk:kk + 1, :])
                wq_sb = cst.tile([128, 8, 128], F16)
                nc.sync.dma_start(out=wq_sb[:],
                                  in_=wq_d.ap().rearrange("(k p) m -> p k m", p=128))
                wkv_sb = cst.tile([128, 8, 128], F16)
                nc.sync.dma_start(out=wkv_sb[:],
                                  in_=wkv_d.ap().rearrange("(k p) m -> p k m", p=128))
                posT_sb = cst.tile([PD, T], F16)
                nc.gpsimd.dma_start(out=posT_sb[:], in_=posT_d.ap())
                wp1T_sb = cst.tile([PD, PD], F16)
                nc.gpsimd.dma_start(out=wp1T_sb[:], in_=wp1T_d.ap())
                wp2T_sb = cst.tile([PD, PF], F16)
                nc.gpsimd.dma_start(out=wp2T_sb[:], in_=wp2T_d.ap())
                whT2_sb = cst.tile([PF, 32], F16)
                nc.gpsimd.dma_start(out=whT2_sb[:], in_=whT2_d.ap())
                bp1_sb = cst.tile([PD, 1], F32)
                nc.sync.dma_start(out=bp1_sb[:], in_=bp1_d.ap())
                bp2_sb = cst.tile([PF, 1], F32)
                nc.sync.dma_start(out=bp2_sb[:], in_=bp2_d.ap())
                g2a_sb = cst.tile([2, 1], F32)
                nc.sync.dma_start(out=g2a_sb[:], in_=g2a_d.ap())
                g2b_sb = cst.tile([33, 1], F32)
                nc.sync.dma_start(out=g2b_sb[:], in_=g2b_d.ap())
                tabc_sb = cst.tile([128, T], F16)
                nc.sync.dma_start(out=tabc_sb[0:32, :], in_=tabc_d.ap())
                for bb in range(1, 4):
                    nc.sync.dma_start(out=tabc_sb[32 * bb:32 * bb + 32, :],
                                      in_=tabc_sb[0:32, :])
                tabs_sb = cst.tile([128, T], F16)
                nc.sync.dma_start(out=tabs_sb[0:64, :], in_=tabs_d.ap())
                nc.sync.dma_start(out=tabs_sb[64:128, :], in_=tabs_sb[0:64, :])

                ones_f = cst.tile([1, 128], F32)
                nc.vector.memset(ones_f, 1.0)
                ones_r = cst.tile([1, 128], F16)
                nc.vector.tensor_scalar_mul(ones_r[:], ones_f[:], 1.0)
                onescol_f = cst.tile([128, 2], F32)
                nc.vector.memset(onescol_f, 1.0)
                onescol_r = cst.tile([128, 2], F16)
                nc.vector.tensor_scalar_mul(onescol_r[:], onescol_f[:], 1.0)
                id64f = cst.tile([64, 64], F32)
                make_identity(nc, id64f)
                id64r = cst.tile([64, 64], F16)
                nc.vector.tensor_scalar_mul(id64r[:], id64f[:], 1.0)

                # ---- q / kv projections (T layout: out rows = proj dims) ----
                qraw = psW.tile([128, T], F32, tag="wide")
                kvraw = psW.tile([128, T], F32, tag="wide")
                for n in range(2):
                    for k in range(8):
                        nc.tensor.matmul(qraw[:, 512 * n:512 * n + 512],
                                         wq_sb[:, k, :],
                                         xT_sb[:, k, 512 * n:512 * n + 512],
                                         start=(k == 0), stop=(k == 7))
                    for k in range(8):
                        nc.tensor.matmul(kvraw[:, 512 * n:512 * n + 512],
                                         wkv_sb[:, k, :],
                                         xT_sb[:, k, 512 * n:512 * n + 512],
                                         start=(k == 0), stop=(k == 7))

                # ---- RoPE on q (rows: [x1_h0, x2_h0, x1_h1, x2_h1] in 32-blocks)
                T1 = wk.tile([128, T], F16)
                T2 = wk.tile([128, T], F16)
                T2s = wk.tile([128, T], F16)
                qT = wk.tile([128, T], F16)
                for n in range(2):
                    c0 = 512 * n
                    nc.vector.tensor_mul(T1[:, c0:c0 + 512], qraw[:, c0:c0 + 512],
                                         tabc_sb[:, c0:c0 + 512])
                    nc.vector.tensor_mul(T2[:, c0:c0 + 512], qraw[:, c0:c0 + 512],
                                         tabs_sb[:, c0:c0 + 512])
                    for b in range(4):
                        sr = (b // 2) * 64 + (1 - (b % 2)) * 32
                        ds = (b // 2) * 64 + (b % 2) * 32
                        nc.sync.dma_start(out=T2s[ds:ds + 32, c0:c0 + 512],
                                          in_=T2[sr:sr + 32, c0:c0 + 512])
                    nc.vector.tensor_add(qT[:, c0:c0 + 512], T1[:, c0:c0 + 512],
                                         T2s[:, c0:c0 + 512])

                # ---- RoPE on k (kvraw rows 0:64) + duplicate into rows 64:128
                T1k = wk.tile([64, T], F16)
                T2k = wk.tile([64, T], F16)
                T2ks = wk.tile([64, T], F16)
                kT2 = wk.tile([128, T], F16)
                for n in range(2):
                    c0 = 512 * n
                    nc.vector.tensor_mul(T1k[:, c0:c0 + 512],
                                         kvraw[0:64, c0:c0 + 512],
                                         tabc_sb[0:64, c0:c0 + 512])
                    nc.vector.tensor_mul(T2k[:, c0:c0 + 512],
                                         kvraw[0:64, c0:c0 + 512],
                                         tabs_sb[0:64, c0:c0 + 512])
                    nc.sync.dma_start(out=T2ks[0:32, c0:c0 + 512],
                                      in_=T2k[32:64, c0:c0 + 512])
                    nc.sync.dma_start(out=T2ks[32:64, c0:c0 + 512],
                                      in_=T2k[0:32, c0:c0 + 512])
                    nc.vector.tensor_add(kT2[0:64, c0:c0 + 512],
                                         T1k[:, c0:c0 + 512],
                                         T2ks[:, c0:c0 + 512])
                    nc.sync.dma_start(out=kT2[64:128, c0:c0 + 512],
                                      in_=kT2[0:64, c0:c0 + 512])

                wo_sb = cst.tile([128, DIM], F16)
                nc.sync.dma_start(out=wo_sb[:], in_=wo_d.ap())

                # ---- v: copy vT out of PSUM, PE-transpose into v_aug (+ones col)
                vT_sb = wk.tile([64, T], F16)
                nc.vector.tensor_copy(vT_sb[:, 0:512], kvraw[64:128, 0:512])
                nc.vector.tensor_copy(vT_sb[:, 512:1024], kvraw[64:128, 512:1024])
                v_aug = []
                for m in range(8):
                    vtp = psS.tile([128, 64], F16, tag="sm")
                    nc.tensor.transpose(vtp[:], vT_sb[:, 128 * m:128 * m + 128],
                                        id64r[:])
                    va = vP.tile([128, 66], F16, tag="vaug")
                    nc.vector.tensor_copy(va[:, 0:64], vtp[:])
                    nc.vector.tensor_copy(va[:, 64:66], onescol_r[:])
                    v_aug.append(va)

                # ---- pos path ----
                pTr = wk.tile([PD, T], F16)
                for n in range(2):
                    pp = psS.tile([PD, 512], F32, tag="sm")
                    nc.tensor.matmul(pp[:], wp1T_sb[:],
                                     posT_sb[:, 512 * n:512 * n + 512],
                                     start=True, stop=True)
                    nc.scalar.activation(pTr[:, 512 * n:512 * n + 512], pp[:],
                                         ReluF, bias=bp1_sb[:, 0:1], scale=1.0)
                p2Tb = wk.tile([PF, T], F16)
                for n in range(2):
                    p2p = psS.tile([PF, 512], F32, tag="sm")
                    nc.tensor.matmul(p2p[:], wp2T_sb[:],
                                     pTr[:, 512 * n:512 * n + 512],
                                     start=True, stop=True)
                    nc.scalar.activation(p2Tb[:, 512 * n:512 * n + 512], p2p[:],
                                         IdentF, bias=bp2_sb[:, 0:1], scale=1.0)
                eposAll = wk.tile([128, 256], F16)
                aALL = psS.tile([128, 256], F32, tag="sm")
                for j in range(8):
                    nc.tensor.matmul(aALL[:, 32 * j:32 * j + 32],
                                     p2Tb[:, 128 * j:128 * j + 128],
                                     whT2_sb[:], start=True, stop=True)
                nc.scalar.activation(eposAll[:], aALL[:], ExpF, scale=-1.0)
                posout = psA.tile([32, 66], F32, tag="av")
                for j in range(8):
                    nc.tensor.matmul(posout[:], eposAll[:, 32 * j:32 * j + 32], v_aug[j][:],
                                     start=(j == 0), stop=(j == 7))
                recipZp = wk.tile([2, 1], F32)
                nc.vector.reciprocal(recipZp[:], posout[0:2, 64:65])
                e2a = wk.tile([2, 1], F32)
                nc.scalar.activation(e2a[:], g2a_sb[:], ExpF, scale=-1.0)
                e2a1 = wk.tile([2, 1], F32)
                nc.vector.tensor_scalar_add(e2a1[:], e2a[:], 1.0)
                sg2a = wk.tile([2, 1], F32)
                nc.vector.reciprocal(sg2a[:], e2a1[:])
                gz2 = wk.tile([2, 1], F32)
                nc.vector.tensor_mul(gz2[:], recipZp[:], sg2a[:])
                gpos2 = wk.tile([2, 64], F16)
                nc.vector.tensor_scalar_mul(gpos2[:], posout[0:2, 0:64], gz2[:, 0:1])
                gposTp = psA.tile([64, 2], F16, tag="av")
                nc.tensor.transpose(gposTp[:], gpos2[:], id64r[0:2, 0:2])
                gposT2 = wk.tile([128, 1], F32)
                nc.vector.tensor_copy(gposT2[0:64, :], gposTp[:, 0:1])
                nc.vector.tensor_copy(gposT2[64:128, :], gposTp[:, 1:2])
                e33 = wk.tile([33, 1], F32)
                nc.scalar.activation(e33[:], g2b_sb[:], ExpF, scale=-1.0)
                e331 = wk.tile([33, 1], F32)
                nc.vector.tensor_scalar_add(e331[:], e33[:], 1.0)
                r331 = wk.tile([33, 1], F32)
                nc.vector.reciprocal(r331[:], e331[:])
                sginv33 = wk.tile([33, 1], F32)
                nc.vector.tensor_mul(sginv33[:], e33[:], r331[:])

                # ---- attention per head ----
                oT = wk.tile([128, T], F16)
                zbS = wk.tile([128, T], F16)
                for i in range(2):
                    r = 64 * i
                    avh = [psA.tile([66, 512], F32, tag="av",
                                    name=f"avh{i}_0"),
                           psA.tile([66, 512], F32, tag="av",
                                    name=f"avh{i}_1")]
                    for m in range(8):
                        E = eP.tile([128, T], F16, tag="E")
                        S = psW.tile([128, T], F32, tag="wide")
                        for n in range(2):
                            nc.tensor.matmul(
                                S[:, 512 * n:512 * n + 512],
                                kT2[r:r + 64, 128 * m:128 * m + 128],
                                qT[r:r + 64, 512 * n:512 * n + 512],
                                start=True, stop=True)
                        nc.scalar.activation(E[:], S[:], ExpF, scale=0.125)
                        for n in range(2):
                            nc.tensor.matmul(avh[n][:],
                                             v_aug[m][:],
                                             E[:, 512 * n:512 * n + 512],
                                             start=(m == 0), stop=(m == 7))
                    for n in range(2):
                        c0 = 512 * n
                        recipZ = wk.tile([1, 512], F32, tag=f"rz{i}{n}")
                        nc.vector.reciprocal(recipZ[:], avh[n][64:65, :])
                        recipZg = wk.tile([1, 512], F16, tag=f"rg{i}{n}")
                        nc.vector.tensor_scalar_mul(
                            recipZg[:], recipZ[:],
                            sginv33[32 * i:32 * i + 1, 0:1])
                        zb = psS.tile([64, 512], F32, tag="sm")
                        nc.tensor.matmul(zb[:], ones_r[:, 0:64],
                                         recipZg[:],
                                         start=True, stop=True)
                        nc.vector.tensor_copy(zbS[r:r + 64, c0:c0 + 512], zb[:])
                        nc.vector.tensor_mul(oT[r:r + 64, c0:c0 + 512],
                                             avh[n][0:64, :],
                                             zbS[r:r + 64, c0:c0 + 512])
                        nc.vector.tensor_scalar_add(
                            oT[r:r + 64, c0:c0 + 512],
                            oT[r:r + 64, c0:c0 + 512],
                            gposT2[r:r + 64, 0:1])

                # ---- output projection (partial over this core's 128 o-cols)
                outp_ap = outp_d.ap()
                for j in range(8):
                    outS = eP.tile([128, DIM], F16, tag="outS")
                    po = psW.tile([128, DIM], F32, tag="wide")
                    for n in range(2):
                        nc.tensor.matmul(po[:, 512 * n:512 * n + 512],
                                         oT[:, 128 * j:128 * j + 128],
                                         wo_sb[:, 512 * n:512 * n + 512],
                                         start=True, stop=True)
                    if j % 2 == 0:
                        nc.scalar.copy(outS[:], po[:])
                    else:
                        nc.vector.tensor_copy(outS[:], po[:])
                    nc.sync.dma_start(
                        out=outp_ap[128 * j:128 * j + 128, :], in_=outS[:])

    nc.compile()
    return nc


def _host_inputs(inputs):
    """Per-core in_maps from the full inputs."""
    x = np.asarray(inputs["x"], np.float32)
    pos = np.asarray(inputs["pos"], np.float32)
    Wq = np.asarray(inputs["Wq"], np.float32)
    Wk = np.asarray(inputs["Wk"], np.float32)
    Wv = np.asarray(inputs["Wv"], np.float32)
    Wo = np.asarray(inputs["Wo"], np.float32)
    bo = np.asarray(inputs["bo"], np.float32)
    Wp1 = np.asarray(inputs["Wp1"], np.float32)
    bp1 = np.asarray(inputs["bp1"], np.float32)
    Wp2 = np.asarray(inputs["Wp2"], np.float32)
    bp2 = np.asarray(inputs["bp2"], np.float32)
    Wh = np.asarray(inputs["Wh"], np.float32)
    gate = np.asarray(inputs["gate"], np.float32)

    xT = np.ascontiguousarray(x[0].T).astype(np.float16)
    posT = np.ascontiguousarray(pos[0].T).astype(np.float16)
    wp1T = np.ascontiguousarray(Wp1.T).astype(np.float16)
    wp2T = np.ascontiguousarray(Wp2.T).astype(np.float16)
    bp1c = bp1.reshape(PD, 1).copy()
    bp2c = bp2.reshape(PF, 1).copy()

    # RoPE tables in transposed layout, tiled 4x along partitions
    j = np.arange(HD // 2, dtype=np.float32)
    theta = (BASE ** (-2.0 * j / HD)).astype(np.float32)
    freqs = np.arange(T, dtype=np.float32)[:, None] * theta  # [T, 32]
    cosT = np.ascontiguousarray(np.cos(freqs).T.astype(np.float32))
    sinT = np.ascontiguousarray(np.sin(freqs).T.astype(np.float32))
    tabc = cosT.astype(np.float16)
    tabs = np.concatenate([sinT, -sinT], 0).astype(np.float16)

    in_maps = []
    for c in range(NC):
        g = c // 2
        wq_c = np.ascontiguousarray(Wq[128 * c:128 * c + 128, :].T).astype(np.float16)
        wkv_c = np.ascontiguousarray(
            np.concatenate([Wk[64 * g:64 * g + 64, :],
                            Wv[64 * g:64 * g + 64, :]], 0).T).astype(np.float16)
        wo_c = np.ascontiguousarray(Wo[:, 128 * c:128 * c + 128].T).astype(np.float16)
        whT2_c = np.zeros((PF, 32), np.float16)
        whT2_c[:, 0:2] = Wh[2 * c:2 * c + 2, :].T
        g2a = gate[2 * c:2 * c + 2].reshape(2, 1).copy()
        g2b = np.zeros((33, 1), np.float32)
        g2b[0, 0] = gate[2 * c]
        g2b[32, 0] = gate[2 * c + 1]
        in_maps.append({
            "xT": xT, "wq": wq_c, "wkv": wkv_c, "wo": wo_c,
            "posT": posT, "wp1T": wp1T, "wp2T": wp2T,
            "whT2": whT2_c, "bp1c": bp1c, "bp2c": bp2c,
            "gate2a": g2a, "gate2b": g2b, "tabc": tabc, "tabs": tabs,
        })
    return in_maps


def get_program(reps=1):
    key = f"nc{reps}"
    if key not in _CACHE:
        _CACHE[key] = _build_program(reps)
    return _CACHE[key]


def kernel(**inputs) -> np.ndarray:
    nc = get_program()
    in_maps = _host_inputs(inputs)
    res = bass_utils.run_bass_kernel_spmd(nc, in_maps, list(range(NC)))
    out = np.zeros((T, DIM), np.float32)
    for c in range(NC):
        out += res.results[c]["outp"].astype(np.float32)
    out += np.asarray(inputs["bo"], np.float32)
    return out.reshape(1, T, DIM)



# revision 2
# speedup vs baseline: 1.3657x; 1.3657x over previous
"""Trainium2 Bass kernel for nn_Attention_62560493633940 — v2.

Sharding Gh=4 x Gq=2: core c handles head-group g = c % 4 (heads 4g..4g+3 =
kv head g) for query-half Q = c // 4 (queries 512Q..512Q+512). Host sums the
4 head-group partials per query half and adds the rank-1 pos rows + bo.

Per-core inputs are rolled along the token axis by QOFF = 512*Q so this
core's queries sit at xT cols 0:512; keys keep the rolled order everywhere
(kv, kT2, va, posT, k RoPE tables) — attention is permutation-invariant over
keys when all key-indexed data is permuted consistently.

Math notes:
- pos_attn is a rank-1 per-head key distribution (shift invariance), so its
  gated output contribution is 1_q (x) (sum_h g_h posv_h) @ Wo — emitted as a
  tiny [1, DIM] `prow` output, added on host. No per-tile broadcast adds.
- (1 - sigmoid(gate_h)) is folded into the zb broadcast matmul lhsT (sgc).
- q RoPE tables are pre-scaled by ALPHA = (2^7/ln2)/8 so S' = ALPHA*S:
  ScalarE exp uses scale = ln2/128; Schraudolph tiles compute
  int16(S' + 16255.5) bitcast to bf16 ~= exp(S/8) within ~3%.
- S matmuls are row-tiled: the 2 heads of a pair sit in partitions 0:64 /
  64:128 of qTs/kT2 -> tile_position (0,0)/(64,0) run concurrently on PE.
"""
import sys

if '/opt/trn_rl_repo' not in sys.path:
    sys.path.insert(0, '/opt/trn_rl_repo')

import numpy as np

import concourse.bass as bass
import concourse.bacc as bacc
import concourse.tile as tile
import concourse.mybir as mybir
from concourse import bass_utils
from concourse.masks import make_identity

F32 = mybir.dt.float32
F16 = mybir.dt.float16
BF16 = mybir.dt.bfloat16
I16 = mybir.dt.int16
F8 = mybir.dt.float8e4

T = 1024
DIM = 1024
H = 16
KVH = 4
HD = 64
PD = 64
PF = 128
BASE = 10000.0
NC = 8
TH = 512
ALPHA = 16.0 / float(np.log(2.0))    # (2^7/ln2)/8
EXP_SCALE = float(np.log(2.0)) / 128.0
# fp8e4(bias 7) Schraudolph on S' = ALPHA*S: bits = 0.0625*S' + (7*8 - 0.5)
SCH_A = 0.0625
SCH_B = 55.5
SCHRAUD = {(0, 3), (0, 6), (1, 2), (1, 5)}

_CACHE = {}


def _build_program(reps=1):
    nc = bacc.Bacc("TRN2")

    xT_d = nc.declare_dram_parameter("xT", [DIM, T], F16, isOutput=False)
    wq_d = nc.declare_dram_parameter("wq", [DIM, 256], F16, isOutput=False)
    wkv_d = nc.declare_dram_parameter("wkv", [DIM, 128], F16, isOutput=False)
    wo_d = nc.declare_dram_parameter("wo", [256, DIM], F16, isOutput=False)
    posT_d = nc.declare_dram_parameter("posT", [PD, T], F16, isOutput=False)
    wp1T_d = nc.declare_dram_parameter("wp1T", [PD, PD], F16, isOutput=False)
    wp2T_d = nc.declare_dram_parameter("wp2T", [PD, PF], F16, isOutput=False)
    whT4_d = nc.declare_dram_parameter("whT4", [PF, 32], F16, isOutput=False)
    bp1_d = nc.declare_dram_parameter("bp1c", [PD, 1], F32, isOutput=False)
    bp2_d = nc.declare_dram_parameter("bp2c", [PF, 1], F32, isOutput=False)
    g4_d = nc.declare_dram_parameter("g4", [4, 1], F32, isOutput=False)
    sgc_d = nc.declare_dram_parameter("sgc", [1, 256], F16, isOutput=False)
    tcq_d = nc.declare_dram_parameter("tcq", [32, TH], F16, isOutput=False)
    tsq_d = nc.declare_dram_parameter("tsq", [64, TH], F16, isOutput=False)
    tck_d = nc.declare_dram_parameter("tck", [32, T], F16, isOutput=False)
    tsk_d = nc.declare_dram_parameter("tsk", [64, T], F16, isOutput=False)
    outp_d = nc.declare_dram_parameter("outp", [TH, DIM], F16, isOutput=True)
    gpos_d = nc.declare_dram_parameter("gpos", [4, 64], F32, isOutput=True)

    ExpF = mybir.ActivationFunctionType.Exp
    ReluF = mybir.ActivationFunctionType.Relu
    IdentF = mybir.ActivationFunctionType.Identity

    with tile.TileContext(nc) as tc, \
         nc.allow_low_precision(reason="output tolerance 2e-2"):
        with tc.tile_pool(name="cst", bufs=2) as cst, \
             tc.tile_pool(name="wk", bufs=2) as wk, \
             tc.tile_pool(name="vP", bufs=16) as vP, \
             tc.tile_pool(name="oS", bufs=2) as oS, \
             tc.tile_pool(name="psW", bufs=2, space="PSUM") as psW, \
             tc.tile_pool(name="psA", bufs=3, space="PSUM") as psA, \
             tc.tile_pool(name="psS", bufs=1, space="PSUM") as psS:

            for _rep in range(reps):
                # ---------------- constants ----------------
                xT_sb = cst.tile([128, 8, T], F16)
                xT_ap = xT_d.ap().rearrange("(k p) t -> p k t", p=128)
                nc.gpsimd.dma_start(out=xT_sb[:, 0:1, 0:512],
                                    in_=xT_ap[:, 0:1, 0:512])
                nc.gpsimd.dma_start(out=xT_sb[:, 0:1, 512:1024],
                                    in_=xT_ap[:, 0:1, 512:1024])
                for kk in range(1, 8):
                    nc.gpsimd.dma_start(out=xT_sb[:, kk:kk + 1, :],
                                        in_=xT_ap[:, kk:kk + 1, :])
                wq_sb = cst.tile([128, 8, 256], F16)
                nc.sync.dma_start(out=wq_sb[:],
                                  in_=wq_d.ap().rearrange("(k p) m -> p k m", p=128))
                wkv_sb = cst.tile([128, 8, 128], F16)
                nc.sync.dma_start(out=wkv_sb[:],
                                  in_=wkv_d.ap().rearrange("(k p) m -> p k m", p=128))
                wo_sb = cst.tile([128, 2, DIM], F16)
                nc.sync.dma_start(out=wo_sb[:],
                                  in_=wo_d.ap().rearrange("(k p) m -> p k m", p=128))
                posT_sb = cst.tile([PD, T], F16)
                nc.gpsimd.dma_start(out=posT_sb[:], in_=posT_d.ap())
                wp1T_sb = cst.tile([PD, PD], F16)
                nc.gpsimd.dma_start(out=wp1T_sb[:], in_=wp1T_d.ap())
                wp2T_sb = cst.tile([PD, PF], F16)
                nc.gpsimd.dma_start(out=wp2T_sb[:], in_=wp2T_d.ap())
                whT4_sb = cst.tile([PF, 32], F16)
                nc.gpsimd.dma_start(out=whT4_sb[:], in_=whT4_d.ap())
                bp1_sb = cst.tile([PD, 1], F32)
                nc.sync.dma_start(out=bp1_sb[:], in_=bp1_d.ap())
                bp2_sb = cst.tile([PF, 1], F32)
                nc.sync.dma_start(out=bp2_sb[:], in_=bp2_d.ap())
                g4_sb = cst.tile([4, 1], F32)
                nc.sync.dma_start(out=g4_sb[:], in_=g4_d.ap())
                sgc_sb = cst.tile([1, 256], F16)
                nc.sync.dma_start(out=sgc_sb[:], in_=sgc_d.ap())

                tcq_sb = cst.tile([128, TH], F16)
                nc.sync.dma_start(out=tcq_sb[0:32, :], in_=tcq_d.ap())
                for bb in range(1, 4):
                    nc.vector.tensor_copy(tcq_sb[32 * bb:32 * bb + 32, :],
                                          tcq_sb[0:32, :])
                tsq_sb = cst.tile([128, TH], F16)
                nc.sync.dma_start(out=tsq_sb[0:64, :], in_=tsq_d.ap())
                nc.vector.tensor_copy(tsq_sb[64:128, :], tsq_sb[0:64, :])
                tck_sb = cst.tile([64, T], F16)
                nc.sync.dma_start(out=tck_sb[0:32, :], in_=tck_d.ap())
                nc.vector.tensor_copy(tck_sb[32:64, :], tck_sb[0:32, :])
                tsk_sb = cst.tile([64, T], F16)
                nc.sync.dma_start(out=tsk_sb[:], in_=tsk_d.ap())

                id64f = cst.tile([64, 64], F32)
                make_identity(nc, id64f)
                id64r = cst.tile([64, 64], F16)
                nc.vector.tensor_scalar_mul(id64r[:], id64f[:], 1.0)

                # ---------------- pos path stage 1 (x-independent) --------
                pTr = wk.tile([PD, T], F16)
                for n in range(2):
                    pp = psS.tile([PD, TH], F32, tag="sm", name=f"pp{n}")
                    nc.tensor.matmul(pp[:], wp1T_sb[:],
                                     posT_sb[:, 512 * n:512 * n + 512],
                                     start=True, stop=True)
                    nc.scalar.activation(pTr[:, 512 * n:512 * n + 512], pp[:],
                                         ReluF, bias=bp1_sb[:, 0:1], scale=1.0)
                p2Tb = wk.tile([PF, T], F16)
                for n in range(2):
                    p2p = psS.tile([PF, TH], F32, tag="sm", name=f"p2p{n}")
                    nc.tensor.matmul(p2p[:], wp2T_sb[:],
                                     pTr[:, 512 * n:512 * n + 512],
                                     start=True, stop=True)
                    nc.scalar.activation(p2Tb[:, 512 * n:512 * n + 512],
                                         p2p[:], IdentF,
                                         bias=bp2_sb[:, 0:1], scale=1.0)
                eposAll = wk.tile([128, 256], F16)
                aALL = psS.tile([128, 256], F32, tag="sm", name="aALL")
                for j in range(8):
                    nc.tensor.matmul(aALL[:, 32 * j:32 * j + 32],
                                     p2Tb[:, 128 * j:128 * j + 128],
                                     whT4_sb[:], start=True, stop=True)
                nc.scalar.activation(eposAll[:], aALL[:], ExpF, scale=-1.0)

                # ---------------- projections + RoPE ----------------
                qraw0 = psW.tile([128, T], F32, tag="wide", name="qraw0")
                for k in range(8):
                    nc.tensor.matmul(qraw0[:, 0:512], wq_sb[:, k, 0:128],
                                     xT_sb[:, k, 0:512],
                                     start=(k == 0), stop=(k == 7))
                kvraw = psW.tile([128, T], F32, tag="wide", name="kvraw")
                for n in range(2):
                    for k in range(8):
                        nc.tensor.matmul(kvraw[:, 512 * n:512 * n + 512],
                                         wkv_sb[:, k, :],
                                         xT_sb[:, k, 512 * n:512 * n + 512],
                                         start=(k == 0), stop=(k == 7))

                # q RoPE pair0
                qTs0 = wk.tile([128, TH], F16)
                T1a = wk.tile([128, TH], F16, name="T1a")
                T2a = wk.tile([128, TH], F16, name="T2a")
                T2as = wk.tile([128, TH], F16, name="T2as")
                nc.vector.tensor_mul(T1a[:], qraw0[:, 0:512], tcq_sb[:])
                nc.vector.tensor_mul(T2a[:], qraw0[:, 0:512], tsq_sb[:])
                for b in range(4):
                    sr = (b // 2) * 64 + (1 - (b % 2)) * 32
                    ds = (b // 2) * 64 + (b % 2) * 32
                    nc.sync.dma_start(out=T2as[ds:ds + 32, :],
                                      in_=T2a[sr:sr + 32, :])
                nc.vector.tensor_add(qTs0[:], T1a[:], T2as[:])

                qraw1 = psW.tile([128, T], F32, tag="wide", name="qraw1")
                for k in range(8):
                    nc.tensor.matmul(qraw1[:, 0:512], wq_sb[:, k, 128:256],
                                     xT_sb[:, k, 0:512],
                                     start=(k == 0), stop=(k == 7))

                # k RoPE + dup rows 64:128 for row-tiling
                kT2 = wk.tile([128, T], F16)
                for n in range(2):
                    c0 = 512 * n
                    T1k = wk.tile([64, TH], F16, name=f"T1k{n}")
                    T2k = wk.tile([64, TH], F16, name=f"T2k{n}")
                    T2ks = wk.tile([64, TH], F16, name=f"T2ks{n}")
                    nc.vector.tensor_mul(T1k[:], kvraw[0:64, c0:c0 + 512],
                                         tck_sb[:, c0:c0 + 512])
                    nc.vector.tensor_mul(T2k[:], kvraw[0:64, c0:c0 + 512],
                                         tsk_sb[:, c0:c0 + 512])
                    nc.sync.dma_start(out=T2ks[0:32, :], in_=T2k[32:64, :])
                    nc.sync.dma_start(out=T2ks[32:64, :], in_=T2k[0:32, :])
                    nc.vector.tensor_add(kT2[0:64, c0:c0 + 512],
                                         T1k[:], T2ks[:])
                    nc.vector.tensor_copy(kT2[64:128, c0:c0 + 512],
                                          kT2[0:64, c0:c0 + 512])

                # q RoPE pair1
                qTs1 = wk.tile([128, TH], F16)
                T1b = wk.tile([128, TH], F16, name="T1b")
                T2b = wk.tile([128, TH], F16, name="T2b")
                T2bs = wk.tile([128, TH], F16, name="T2bs")
                nc.vector.tensor_mul(T1b[:], qraw1[:, 0:512], tcq_sb[:])
                nc.vector.tensor_mul(T2b[:], qraw1[:, 0:512], tsq_sb[:])
                for b in range(4):
                    sr = (b // 2) * 64 + (1 - (b % 2)) * 32
                    ds = (b // 2) * 64 + (b % 2) * 32
                    nc.sync.dma_start(out=T2bs[ds:ds + 32, :],
                                      in_=T2b[sr:sr + 32, :])
                nc.vector.tensor_add(qTs1[:], T1b[:], T2bs[:])

                # v: PSUM -> SBUF, transpose chunks into va tiles.
                # va16: fp16, per chunk (pos AV); va8: fp8 m-pair-interleaved
                # [128, 2, 66] for the DoubleRow content AV.
                vT_sb = wk.tile([64, T], F16)
                nc.scalar.copy(vT_sb[:, 0:512], kvraw[64:128, 0:512])
                nc.scalar.copy(vT_sb[:, 512:1024], kvraw[64:128, 512:1024])
                va = []
                va8 = []
                for t_ in range(4):
                    # pair-dim stride must be a multiple of 16B for DoubleRow
                    v8 = vP.tile([128, 2, 80], F8, tag="v8", name=f"va8_{t_}")
                    nc.vector.memset(v8[:, :, 64:66], 1.0)
                    va8.append(v8)
                for m in range(8):
                    vtp = psS.tile([128, 64], F16, tag="sm", name=f"vtp{m}")
                    nc.tensor.transpose(vtp[:], vT_sb[:, 128 * m:128 * m + 128],
                                        id64r[:])
                    vam = vP.tile([128, 66], F16, tag="vaug")
                    nc.vector.tensor_copy(vam[:, 0:64], vtp[:])
                    nc.vector.memset(vam[:, 64:66], 1.0)
                    va.append(vam)
                    nc.vector.tensor_copy(va8[m // 2][:, m % 2, 0:64], vtp[:])

                # pos AV + posv marshal + prow
                posout = psA.tile([32, 66], F32, tag="av", name="posout")
                for j in range(8):
                    nc.tensor.matmul(posout[:], eposAll[:, 32 * j:32 * j + 32],
                                     va[j][:], start=(j == 0), stop=(j == 7))
                recipZp = wk.tile([4, 1], F32)
                nc.vector.reciprocal(recipZp[:], posout[0:4, 64:65])
                e4 = wk.tile([4, 1], F32)
                nc.scalar.activation(e4[:], g4_sb[:], ExpF, scale=-1.0)
                e41 = wk.tile([4, 1], F32)
                nc.vector.tensor_scalar_add(e41[:], e4[:], 1.0)
                sig4 = wk.tile([4, 1], F32)
                nc.vector.reciprocal(sig4[:], e41[:])
                gz4 = wk.tile([4, 1], F32)
                nc.vector.tensor_mul(gz4[:], recipZp[:], sig4[:])
                gpos2 = wk.tile([4, 64], F32)
                nc.vector.tensor_scalar_mul(gpos2[:], posout[0:4, 0:64],
                                            gz4[:, 0:1])
                nc.sync.dma_start(out=gpos_d.ap(), in_=gpos2[:])

                # ---------------- attention (two head-pair phases) --------
                oTs0 = wk.tile([128, TH], F16)
                oTs1 = wk.tile([128, TH], F16)
                DR = mybir.MatmulPerfMode.DoubleRow
                for p, (qTs, oTs) in enumerate(((qTs0, oTs0), (qTs1, oTs1))):
                    avh = [psA.tile([66, TH], F32, tag="av",
                                    name=f"avh{p}_{i}") for i in range(2)]
                    E_ring = wk.tile([128, 4, T], F8, name=f"ering{p}")
                    for m in range(8):
                        S2 = psW.tile([128, T], F32, tag="wide")
                        nc.tensor.matmul(S2[:, 0:512],
                                         kT2[0:64, 128 * m:128 * m + 128],
                                         qTs[0:64, :], start=True, stop=True)
                        nc.tensor.matmul(S2[:, 512:1024],
                                         kT2[64:128, 128 * m:128 * m + 128],
                                         qTs[64:128, :], start=True, stop=True)
                        if (p, m) in SCHRAUD:
                            nc.vector.tensor_scalar(
                                E_ring.bitcast(mybir.dt.uint8)[:, m % 4, :],
                                S2[:], SCH_A, SCH_B,
                                mybir.AluOpType.mult, mybir.AluOpType.add)
                        else:
                            nc.scalar.activation(E_ring[:, m % 4, :], S2[:],
                                                 ExpF, scale=EXP_SCALE)
                        if m % 2 == 1:
                            t_ = m // 2
                            s0 = (m - 1) % 4
                            for i in range(2):
                                Epair = E_ring[:, s0:s0 + 2,
                                               512 * i:512 * i + 512]
                                nc.tensor.matmul(
                                    avh[i][:], va8[t_][:, :, 0:66], Epair,
                                    start=(t_ == 0), stop=(t_ == 3),
                                    perf_mode=DR)
                    zb = psS.tile([128, TH], F32, tag="sm", name=f"zb{p}")
                    for i in range(2):
                        h = 2 * p + i
                        rz = wk.tile([1, TH], F16, name=f"rz{h}")
                        nc.vector.reciprocal(rz[:], avh[i][64:65, :])
                        nc.tensor.matmul(zb[64 * i:64 * i + 64, :],
                                         sgc_sb[0:1, 64 * h:64 * h + 64],
                                         rz[:], start=True, stop=True)
                    zbS = wk.tile([128, TH], F16, name=f"zbS{p}")
                    nc.scalar.copy(zbS[:], zb[:])
                    for i in range(2):
                        nc.vector.tensor_mul(oTs[64 * i:64 * i + 64, :],
                                             avh[i][0:64, :],
                                             zbS[64 * i:64 * i + 64, :])

                # ---------------- output projection ----------------
                outp_ap = outp_d.ap()
                for j in range(4):
                    po = psW.tile([128, DIM], F32, tag="wide")
                    for n in range(2):
                        nc.tensor.matmul(po[:, 512 * n:512 * n + 512],
                                         oTs0[:, 128 * j:128 * j + 128],
                                         wo_sb[:, 0, 512 * n:512 * n + 512],
                                         start=True, stop=False)
                        nc.tensor.matmul(po[:, 512 * n:512 * n + 512],
                                         oTs1[:, 128 * j:128 * j + 128],
                                         wo_sb[:, 1, 512 * n:512 * n + 512],
                                         start=False, stop=True)
                    outS = oS.tile([128, DIM], F16, tag="outS")
                    if j % 2 == 0:
                        nc.scalar.copy(outS[:], po[:])
                    else:
                        nc.vector.tensor_copy(outS[:], po[:])
                    nc.sync.dma_start(out=outp_ap[128 * j:128 * j + 128, :],
                                      in_=outS[:])

    nc.compile()
    return nc


def _host_inputs(inputs):
    """Per-core in_maps. Core c: head-group g = c % 4, query-half Q = c // 4.
    Token axis rolled by QOFF = 512*Q (queries first, keys consistent)."""
    x = np.asarray(inputs["x"], np.float32)[0]
    pos = np.asarray(inputs["pos"], np.float32)[0]
    Wq = np.asarray(inputs["Wq"], np.float32)
    Wk = np.asarray(inputs["Wk"], np.float32)
    Wv = np.asarray(inputs["Wv"], np.float32)
    Wo = np.asarray(inputs["Wo"], np.float32)
    Wp1 = np.asarray(inputs["Wp1"], np.float32)
    bp1 = np.asarray(inputs["bp1"], np.float32)
    Wp2 = np.asarray(inputs["Wp2"], np.float32)
    bp2 = np.asarray(inputs["bp2"], np.float32)
    Wh = np.asarray(inputs["Wh"], np.float32)
    gate = np.asarray(inputs["gate"], np.float32)

    wp1T = np.ascontiguousarray(Wp1.T).astype(np.float16)
    wp2T = np.ascontiguousarray(Wp2.T).astype(np.float16)
    bp1c = bp1.reshape(PD, 1).astype(np.float32)
    bp2c = bp2.reshape(PF, 1).astype(np.float32)

    jj = np.arange(HD // 2, dtype=np.float32)
    theta = (BASE ** (-2.0 * jj / HD)).astype(np.float32)
    freqs = np.arange(T, dtype=np.float32)[:, None] * theta   # [T, 32]
    cosT = np.cos(freqs).T.astype(np.float32)                 # [32, T]
    sinT = np.sin(freqs).T.astype(np.float32)

    in_maps = []
    for c in range(NC):
        g = c % 4
        Q = c // 4
        qoff = TH * Q
        roll = np.concatenate([np.arange(qoff, T), np.arange(0, qoff)])
        xr = x[roll]                       # [T, DIM] rolled tokens
        posr = pos[roll]
        xT = np.ascontiguousarray(xr.T).astype(np.float16)
        posT = np.ascontiguousarray(posr.T).astype(np.float16)

        wq_c = np.ascontiguousarray(
            Wq[256 * g:256 * g + 256, :].T).astype(np.float16)      # [DIM, 256]
        wkv_c = np.ascontiguousarray(
            np.concatenate([Wk[64 * g:64 * g + 64, :],
                            Wv[64 * g:64 * g + 64, :]], 0).T).astype(np.float16)
        wo_c = np.ascontiguousarray(
            Wo[:, 256 * g:256 * g + 256].T).astype(np.float16)      # [256, DIM]
        whT4_c = np.zeros((PF, 32), np.float16)
        whT4_c[:, 0:4] = Wh[4 * g:4 * g + 4, :].T
        g4 = gate[4 * g:4 * g + 4].reshape(4, 1).astype(np.float32)
        sig = 1.0 / (1.0 + np.exp(-gate[4 * g:4 * g + 4]))
        sgc = np.zeros((1, 256), np.float16)
        for i in range(4):
            sgc[0, 64 * i:64 * i + 64] = (1.0 - sig[i])

        # q tables: absolute query positions qoff..qoff+511, ALPHA-scaled
        qpos = (np.arange(qoff, qoff + TH)) % T
        tcq = (ALPHA * cosT[:, qpos]).astype(np.float16)            # [32, TH]
        tsq = np.concatenate([ALPHA * sinT[:, qpos],
                              -ALPHA * sinT[:, qpos]], 0).astype(np.float16)
        # k tables: rolled full sequence
        tck = cosT[:, roll].astype(np.float16)                      # [32, T]
        tsk = np.concatenate([sinT[:, roll], -sinT[:, roll]],
                             0).astype(np.float16)                  # [64, T]

        in_maps.append({
            "xT": xT, "wq": wq_c, "wkv": wkv_c, "wo": wo_c,
            "posT": posT, "wp1T": wp1T, "wp2T": wp2T, "whT4": whT4_c,
            "bp1c": bp1c, "bp2c": bp2c, "g4": g4, "sgc": sgc,
            "tcq": tcq, "tsq": tsq, "tck": tck, "tsk": tsk,
        })
    return in_maps


def get_program(reps=1):
    key = f"nc{reps}"
    if key not in _CACHE:
        _CACHE[key] = _build_program(reps)
    return _CACHE[key]


def kernel(**inputs) -> np.ndarray:
    nc = get_program()
    in_maps = _host_inputs(inputs)
    res = bass_utils.run_bass_kernel_spmd(nc, in_maps, list(range(NC)))
    Wo = np.asarray(inputs["Wo"], np.float32)
    out = np.zeros((T, DIM), np.float32)
    for c in range(NC):
        g = c % 4
        Q = c // 4
        r0 = TH * Q
        out[r0:r0 + TH] += res.results[c]["outp"].astype(np.float32)
        gp = res.results[c]["gpos"].astype(np.float32)
        prow = np.zeros(DIM, np.float32)
        for i in range(4):
            h = 4 * g + i
            prow += gp[i] @ Wo[:, 64 * h:64 * h + 64].T
        out[r0:r0 + TH] += prow[None, :]
    out += np.asarray(inputs["bo"], np.float32)
    return out.reshape(1, T, DIM)
